# Initial kernel scaffold
#
"""Trainium2 Bass kernel for a dense transformer block (pre-norm attention +
GeGLU MLP), tensor-parallel across 8 NeuronCores.

Sharding: core c owns heads [2c, 2c+1] (wq/wk/wv column-shards, wo row-shard)
and GeGLU hidden slice [c*1024,(c+1)*1024) (w1 column-, w2 row-parallel).
Norms/residuals are sequence-parallel: core r owns rows {st*512 + r*64 + i}
for st in 0..3 (aligned to the ReduceScatter chunking).

Pipelined collectives, all chunked so they overlap compute:
  AllGather(y1T) x4 feature-groups -> QKV/attention TP ->
  ReduceScatter(wo partials) x4 row-chunks -> local norm2 per chunk ->
  AllGather(y2T) x4 -> MLP TP per quarter -> ReduceScatter(w2 partials) x4
  -> local residual add -> host gathers row blocks.

All matmuls run in float32r (TF32-like, ~1.5e-4 rel err, full PE rate at
moving-dim >= 256). Layouts are feature-major ("T") so every matmul
contraction sits on the partition axis. Softmax is computed with j (keys)
on partitions: exp on ACT, row-sums via ones-matmul, reciprocal broadcast
back over partitions with a K=1 matmul.
"""

import sys

for _p in ("/opt/trn_rl_repo",):
    if _p not in sys.path:
        sys.path.insert(0, _p)

import math
from dataclasses import dataclass

import numpy as np


@dataclass(frozen=True)
class Cfg:
    S: int = 2048       # sequence length
    D: int = 2048       # model dim
    H: int = 16         # heads (total)
    DH: int = 128       # head dim (must be 128)
    HID: int = 8192     # GeGLU hidden (total)
    NC: int = 8         # cores
    EPS: float = 1e-5

    @property
    def P(self):
        return 128

    @property
    def SL(self):   # rows per core
        return self.S // self.NC

    @property
    def SLT(self):  # local row tiles of 128
        return self.SL // self.P

    @property
    def DC(self):   # d chunks of 128
        return self.D // self.P

    @property
    def MH(self):   # heads per core
        return self.H // self.NC

    @property
    def ML(self):   # local qkv features
        return self.MH * self.DH

    @property
    def HL(self):   # local hidden
        return self.HID // self.NC

    @property
    def HLT(self):  # local hidden tiles of 128
        return self.HL // self.P

    @property
    def ST(self):   # sequence tiles of 512 == row-chunk count
        return self.S // 512

    @property
    def RW(self):   # rows per (chunk, rank)
        return 512 // self.NC

    @property
    def CPT(self):  # chunks per local 128-row tile
        return self.P // self.RW

    @property
    def DG(self):   # AG1 feature groups
        return 4

    @property
    def GD(self):   # d-chunks per AG1 group
        return self.DC // self.DG


FULL = Cfg()

_BUILT = {}


def _build(cfg: Cfg):
    """Build + compile the SPMD program."""
    import concourse.tile as tile
    from concourse import bacc, mybir
    from concourse.masks import make_identity

    P = cfg.P
    F32 = mybir.dt.float32
    F32R = mybir.dt.float32r
    assert cfg.DH == P and cfg.ML == 256 and cfg.S % 512 == 0

    nc = bacc.Bacc("TRN2", target_bir_lowering=False, debug=False,
                   num_devices=cfg.NC)

    def din(name, shape, dt=F32):
        return nc.dram_tensor(name, list(shape), dt, kind="ExternalInput").ap()

    x_loc = din("x_loc", [cfg.SL, cfg.D])
    wqT = din("wqT", [cfg.D, cfg.ML], F32R)
    wkT = din("wkT", [cfg.D, cfg.ML], F32R)
    wvT = din("wvT", [cfg.D, cfg.ML], F32R)
    woT = din("woT", [cfg.ML, cfg.D], F32R)
    w1hT = din("w1hT", [cfg.D, cfg.HL], F32R)
    w1gT = din("w1gT", [cfg.D, cfg.HL], F32R)
    w2T = din("w2T", [cfg.HL, cfg.D], F32R)
    b1h = din("b1h", [cfg.HL])
    b1g = din("b1g", [cfg.HL])
    b2 = din("b2", [cfg.D], F32R)
    n1w = din("n1w", [cfg.D])
    n1b = din("n1b", [cfg.D])
    n2w = din("n2w", [cfg.D])
    n2b = din("n2b", [cfg.D])

    out_loc = nc.dram_tensor("out_loc", [cfg.SL, cfg.D], F32,
                             kind="ExternalOutput").ap()

    rg = [list(range(cfg.NC))]
    AX = mybir.AxisListType.X
    ALU = mybir.AluOpType
    ACTF = mybir.ActivationFunctionType
    inv_sqrt_dh = 1.0 / math.sqrt(cfg.DH)

    with tile.TileContext(nc) as tc:
        # ---- internal DRAM ----
        dram = tc.alloc_tile_pool(name="dram", bufs=1, space="DRAM")
        y1t_loc = [dram.tile([cfg.GD * P, cfg.SL], F32R, name=f"y1t_loc{g}")
                   for g in range(cfg.DG)]
        y1t_ag = [dram.tile([cfg.NC, cfg.GD * P, cfg.SL], F32R,
                            name=f"y1t_ag{g}", addr_space="Shared")
                  for g in range(cfg.DG)]
        part_o = [dram.tile([512, cfg.D], F32, name=f"part_o{c}")
                  for c in range(cfg.ST)]
        rs1 = [dram.tile([cfg.RW, cfg.D], F32, name=f"rs1_{c}")
               for c in range(cfg.ST)]
        y2t_loc = [dram.tile([cfg.D, cfg.RW], F32R, name=f"y2t_loc{c}")
                   for c in range(cfg.ST)]
        y2t_ag = [dram.tile([cfg.NC, cfg.D, cfg.RW], F32R,
                            name=f"y2t_ag{c}", addr_space="Shared")
                  for c in range(cfg.ST)]
        part_2 = [dram.tile([512, cfg.D], F32, name=f"part_2_{c}")
                  for c in range(cfg.ST)]
        rs2 = [dram.tile([cfg.RW, cfg.D], F32, name=f"rs2_{c}")
               for c in range(cfg.ST)]

        # ---- constants / persistent small tiles ----
        consts = tc.alloc_tile_pool(name="consts", bufs=1)
        ident = consts.tile([P, P], F32, name="ident")
        make_identity(nc, ident)
        ones_f32 = consts.tile([P, 1], F32, name="ones_f32")
        nc.vector.memset(ones_f32, 1.0)
        ones_row_f32 = consts.tile([1, P], F32, name="ones_row_f32")
        nc.vector.memset(ones_row_f32, 1.0)
        ones_col = consts.tile([P, 1], F32R, name="ones_col")
        nc.vector.tensor_copy(out=ones_col, in_=ones_f32)
        ones_row = consts.tile([1, P], F32R, name="ones_row")
        nc.vector.tensor_copy(out=ones_row, in_=ones_row_f32)
        eps_t = consts.tile([P, 1], F32, name="eps_t")
        nc.vector.memset(eps_t, cfg.EPS)

        def load_pp(name, src, n):  # [n*P] dram -> [P, n] sbuf (per-partition)
            t = consts.tile([P, n], F32, name=name)
            nc.sync.dma_start(out=t, in_=src.rearrange("(t p) -> p t", p=P))
            return t

        n1w_t = load_pp("n1w_t", n1w, cfg.DC)
        n1b_t = load_pp("n1b_t", n1b, cfg.DC)
        n2w_t = load_pp("n2w_t", n2w, cfg.DC)
        n2b_t = load_pp("n2b_t", n2b, cfg.DC)
        b1h_t = load_pp("b1h_t", b1h, cfg.HLT)
        b1g_t = load_pp("b1g_t", b1g, cfg.HLT)

        b2row = consts.tile([1, cfg.D], F32R, name="b2row")
        nc.sync.dma_start(out=b2row, in_=b2.rearrange("(o d) -> o d", o=1))
        b2bc = consts.tile([P, cfg.D], F32, name="b2bc")
        with tc.tile_pool(name="ps_init", bufs=1, space="PSUM") as ps_init:
            for i in range(cfg.D // 512):
                pb = ps_init.tile([P, 512], F32, name="pb")
                nc.tensor.matmul(pb, ones_row, b2row[:, i * 512:(i + 1) * 512],
                                 start=True, stop=True)
                nc.vector.tensor_copy(out=b2bc[:, i * 512:(i + 1) * 512], in_=pb)

        # residuals (per row-chunk, base partition 0)
        x2res = tc.alloc_tile_pool(name="x2res", bufs=1)
        x2c = [x2res.tile([cfg.RW, cfg.D], F32, name=f"x2c{c}")
               for c in range(cfg.ST)]
        xres = tc.alloc_tile_pool(name="xres", bufs=1)
        xc = [xres.tile([cfg.RW, cfg.D], F32, name=f"xc{c}")
              for c in range(cfg.ST)]

        # ---- norm helpers ----
        def norm_rows(src_ap, rw, xn_pool, tmp_pool, tag):
            """rmsnorm of [rw, D] rows -> normalized tile (pre scale/bias).
            One [rw, D] tile is used for x^2 then overwritten with x*inv."""
            xn = xn_pool.tile([rw, cfg.D], F32, name=f"{tag}_xn",
                              tag=f"{tag}_xn")
            nc.vector.tensor_mul(xn, src_ap, src_ap)
            ssum = tmp_pool.tile([rw, 1], F32, name="nrm_ss", tag="nrm_ss")
            nc.vector.tensor_reduce(out=ssum, in_=xn, axis=AX, op=ALU.add)
            rms = tmp_pool.tile([rw, 1], F32, name="nrm_rms", tag="nrm_rms")
            nc.scalar.activation(out=rms, in_=ssum, func=ACTF.Sqrt,
                                 bias=eps_t[:rw], scale=1.0 / cfg.D)
            inv = tmp_pool.tile([rw, 1], F32, name="nrm_inv", tag="nrm_inv")
            nc.vector.reciprocal(out=inv, in_=rms)
            nc.vector.tensor_scalar_mul(xn, src_ap, inv)
            return xn

        def transpose_scaled(xn, rw, d, w_t, b_t, dst_ap, pool, psp, tag):
            """transpose xn[:, d-slice] ([rw,128]) -> [128,rw], apply
            per-feature w/b, DMA to dst_ap ([128, rw] in DRAM)."""
            pT = psp.tile([P, rw], F32, name=f"{tag}_pT", tag=f"{tag}_pT")
            nc.tensor.transpose(pT, xn[:, d * P:(d + 1) * P],
                                ident[:rw, :rw])
            yt = pool.tile([P, rw], F32R, name=f"{tag}_yt", tag=f"{tag}_yt")
            nc.vector.tensor_scalar(out=yt, in0=pT,
                                    scalar1=w_t[:, d:d + 1],
                                    scalar2=b_t[:, d:d + 1],
                                    op0=ALU.mult, op1=ALU.add)
            nc.sync.dma_start(out=dst_ap, in_=yt)

        # ================= phase 0: norm1 + grouped AG =================
        with tc.tile_pool(name="nrm1", bufs=2) as pool, \
             tc.tile_pool(name="nrm1x", bufs=1) as xnpool, \
             tc.tile_pool(name="nrm1_ps", bufs=2, space="PSUM") as psp:
            xn1 = []
            for c in range(cfg.ST):
                nc.sync.dma_start(out=xc[c],
                                  in_=x_loc[c * cfg.RW:(c + 1) * cfg.RW, :])
                xn1.append(norm_rows(xc[c], cfg.RW, xnpool, pool, f"n1_{c}"))
            for g in range(cfg.DG):
                for c in range(cfg.ST):
                    for dd in range(cfg.GD):
                        d = g * cfg.GD + dd
                        transpose_scaled(
                            xn1[c], cfg.RW, d, n1w_t, n1b_t,
                            y1t_loc[g][dd * P:(dd + 1) * P,
                                       c * cfg.RW:(c + 1) * cfg.RW],
                            pool, psp, "n1")
                nc.gpsimd.collective_compute(
                    "AllGather", ALU.bypass, replica_groups=rg,
                    ins=[y1t_loc[g][:]], outs=[y1t_ag[g][:]])

        # persistent qkv results
        qkvres = tc.alloc_tile_pool(name="qkvres", bufs=1)
        qT = [qkvres.tile([P, cfg.S], F32R, name=f"qT{m}")
              for m in range(cfg.MH)]
        kT = [qkvres.tile([P, cfg.S], F32R, name=f"kT{m}")
              for m in range(cfg.MH)]
        v_sb = [qkvres.tile([P, cfg.ML], F32R, name=f"v{j}")
                for j in range(cfg.S // P)]

        def y1_tile_dma(y1, d, st):
            g, dd = d // cfg.GD, d % cfg.GD
            nc.sync.dma_start(
                out=y1.rearrange("p (b s) -> p b s", b=cfg.NC),
                in_=y1t_ag[g][:, dd * P:(dd + 1) * P,
                              st * cfg.RW:(st + 1) * cfg.RW]
                .rearrange("b d s -> d b s"))

        # ================= phase 1: QKV =================
        with tc.tile_pool(name="qkv_w", bufs=1) as wpool, \
             tc.tile_pool(name="qkv_s", bufs=3) as spool, \
             tc.tile_pool(name="qkv_pq", bufs=1, space="PSUM") as pq, \
             tc.tile_pool(name="qkv_pk", bufs=1, space="PSUM") as pk, \
             tc.tile_pool(name="qkv_pv", bufs=1, space="PSUM") as pv:
            wq_t = wpool.tile([P, cfg.DC, cfg.ML], F32R, name="wq_t")
            wk_t = wpool.tile([P, cfg.DC, cfg.ML], F32R, name="wk_t")
            wv_t = wpool.tile([P, cfg.DC, cfg.ML], F32R, name="wv_t")
            for w_t, src in ((wq_t, wqT), (wk_t, wkT), (wv_t, wvT)):
                nc.sync.dma_start(
                    out=w_t, in_=src.rearrange("(c p) m -> p c m", p=P))
            for st in range(cfg.ST):
                q_ps = [pq.tile([P, 512], F32, name=f"q_ps{m}")
                        for m in range(cfg.MH)]
                k_ps = [pk.tile([P, 512], F32, name=f"k_ps{m}")
                        for m in range(cfg.MH)]
                v_ps = [pv.tile([P, cfg.ML], F32, name=f"v_ps{j}")
                        for j in range(4)]
                for d in range(cfg.DC):
                    y1 = spool.tile([P, 512], F32R, name="y1s")
                    y1_tile_dma(y1, d, st)
                    first, last = d == 0, d == cfg.DC - 1
                    for m in range(cfg.MH):
                        nc.tensor.matmul(
                            q_ps[m], wq_t[:, d, m * P:(m + 1) * P], y1,
                            start=first, stop=last)
                        nc.tensor.matmul(
                            k_ps[m], wk_t[:, d, m * P:(m + 1) * P], y1,
                            start=first, stop=last)
                    for ss in range(4):
                        nc.tensor.matmul(
                            v_ps[ss], y1[:, ss * P:(ss + 1) * P],
                            wv_t[:, d, :], start=first, stop=last)
                for m in range(cfg.MH):
                    nc.vector.tensor_copy(
                        out=qT[m][:, st * 512:(st + 1) * 512], in_=q_ps[m])
                    nc.vector.tensor_copy(
                        out=kT[m][:, st * 512:(st + 1) * 512], in_=k_ps[m])
                for ss in range(4):
                    nc.vector.tensor_copy(
                        out=v_sb[st * 4 + ss], in_=v_ps[ss])

        # persistent attention output (feature-major, per head)
        att_res = tc.alloc_tile_pool(name="att_res", bufs=1)
        aoT = [att_res.tile([P, cfg.S], F32R, name=f"aoT{m}")
               for m in range(cfg.MH)]
        woT_t = [att_res.tile([P, cfg.D], F32R, name=f"woT{m}")
                 for m in range(cfg.MH)]
        for m in range(cfg.MH):
            nc.sync.dma_start(out=woT_t[m], in_=woT[m * P:(m + 1) * P, :])

        # ======== phases 2+3: attention + wo + chunked RS1/norm2/AG2 ======
        JT = cfg.S // P

        def phase3_chunk(c, pool, smpool, psp):
            rw = cfg.RW
            r1 = pool.tile([rw, cfg.D], F32, name="r1", tag="r1")
            nc.sync.dma_start(out=r1, in_=rs1[c][:])
            nc.vector.tensor_add(x2c[c], xc[c], r1)
            xn2 = norm_rows(x2c[c], rw, pool, smpool, "n2")
            for d in range(cfg.DC):
                transpose_scaled(xn2, rw, d, n2w_t, n2b_t,
                                 y2t_loc[c][d * P:(d + 1) * P, :],
                                 smpool, psp, "n2")
            nc.gpsimd.collective_compute(
                "AllGather", ALU.bypass, replica_groups=rg,
                ins=[y2t_loc[c][:]], outs=[y2t_ag[c][:]])

        with tc.tile_pool(name="att_s", bufs=4) as expool, \
             tc.tile_pool(name="att_m", bufs=2) as spool, \
             tc.tile_pool(name="nrm2big", bufs=1) as n2pool, \
             tc.tile_pool(name="nrm2sm", bufs=2) as n2sm, \
             tc.tile_pool(name="att_pqk", bufs=2, space="PSUM") as pqk, \
             tc.tile_pool(name="att_pav", bufs=2, space="PSUM") as pav, \
             tc.tile_pool(name="att_psb", bufs=1, space="PSUM") as psb, \
             tc.tile_pool(name="att_ppo", bufs=1, space="PSUM") as ppo, \
             tc.tile_pool(name="nrm2_ps", bufs=2, space="PSUM") as psp2:
            for st in range(cfg.ST):
                sl = slice(st * 512, (st + 1) * 512)
                for h in range(cfg.MH):
                    av_ps = pav.tile([P, 512], F32, name="av_ps")
                    sum_ps = psb.tile([1, 512], F32, name="sum_ps",
                                      tag="smbc")
                    for j in range(JT):
                        qk_ps = pqk.tile([P, 512], F32, name="qk_ps")
                        nc.tensor.matmul(qk_ps, kT[h][:, j * P:(j + 1) * P],
                                         qT[h][:, sl], start=True, stop=True)
                        ex = expool.tile([P, 512], F32R, name="ex")
                        nc.scalar.activation(out=ex, in_=qk_ps, func=ACTF.Exp,
                                             scale=inv_sqrt_dh)
                        nc.tensor.matmul(sum_ps, ones_col, ex,
                                         start=(j == 0), stop=(j == JT - 1))
                        nc.tensor.matmul(av_ps,
                                         v_sb[j][:, h * P:(h + 1) * P], ex,
                                         start=(j == 0), stop=(j == JT - 1))
                    rec = spool.tile([1, 512], F32R, name="rec")
                    with nc.allow_low_precision(
                            reason="softmax denom reciprocal in f32r"):
                        nc.vector.reciprocal(out=rec, in_=sum_ps)
                    bc_ps = psb.tile([P, 512], F32, name="bc_ps", tag="smbc")
                    nc.tensor.matmul(bc_ps, ones_row, rec,
                                     start=True, stop=True)
                    bc_sb = spool.tile([P, 512], F32, name="bc_sb")
                    nc.vector.tensor_copy(out=bc_sb, in_=bc_ps)
                    nc.vector.tensor_mul(aoT[h][:, sl], av_ps, bc_sb)
                # wo for this s-range
                for ss in range(4):
                    s0 = st * 512 + ss * P
                    for dt in range(cfg.D // 512):
                        po_ps = ppo.tile([P, 512], F32, name="po_ps")
                        for m in range(cfg.MH):
                            nc.tensor.matmul(
                                po_ps, aoT[m][:, s0:s0 + P],
                                woT_t[m][:, dt * 512:(dt + 1) * 512],
                                start=(m == 0), stop=(m == cfg.MH - 1))
                        po_sb = spool.tile([P, 512], F32, name="po_sb")
                        nc.vector.tensor_copy(out=po_sb, in_=po_ps)
                        nc.sync.dma_start(
                            out=part_o[st][ss * P:(ss + 1) * P,
                                           dt * 512:(dt + 1) * 512],
                            in_=po_sb)
                nc.gpsimd.collective_compute(
                    "ReduceScatter", ALU.add, replica_groups=rg,
                    ins=[part_o[st][:]], outs=[rs1[st][:]])
                if st >= 1:
                    phase3_chunk(st - 1, n2pool, n2sm, psp2)
            phase3_chunk(cfg.ST - 1, n2pool, n2sm, psp2)
        att_res.release()
        qkvres.release()
        xres.release()

        # ============ phase 4: MLP (per quarter) + RS2 + final ============
        def final_chunk(c, pool):
            rw = cfg.RW
            r2 = pool.tile([rw, cfg.D], F32, name="r2", tag="r2")
            nc.sync.dma_start(out=r2, in_=rs2[c][:])
            nc.vector.tensor_add(r2, r2, b2bc[:rw])
            o_t = pool.tile([rw, cfg.D], F32, name="o_t", tag="o_t")
            nc.vector.tensor_add(o_t, r2, x2c[c])
            nc.sync.dma_start(out=out_loc[c * rw:(c + 1) * rw, :], in_=o_t)

        NQ = cfg.ST
        QW = 512
        with tc.tile_pool(name="mlp_y", bufs=1) as ypool, \
             tc.tile_pool(name="mlp_u", bufs=1) as upool, \
             tc.tile_pool(name="mlp_w", bufs=2) as wpool, \
             tc.tile_pool(name="mlp_s", bufs=3) as spool, \
             tc.tile_pool(name="fin", bufs=1) as fpool, \
             tc.tile_pool(name="mlp_ph", bufs=2, space="PSUM") as ph, \
             tc.tile_pool(name="mlp_pg", bufs=2, space="PSUM") as pg, \
             tc.tile_pool(name="mlp_p2", bufs=3, space="PSUM") as p2:
            for qi in range(NQ):
                y2q = [ypool.tile([P, QW], F32R, name=f"y2q{d}",
                                  tag=f"y2q{d}")
                       for d in range(cfg.DC)]
                for d in range(cfg.DC):
                    nc.sync.dma_start(
                        out=y2q[d].rearrange("p (b s) -> p b s", b=cfg.NC),
                        in_=y2t_ag[qi][:, d * P:(d + 1) * P, :]
                        .rearrange("b d s -> d b s"))
                uT = [upool.tile([P, QW], F32R, name=f"uT{m}", tag=f"uT{m}")
                      for m in range(cfg.HLT)]
                for mt in range(cfg.HLT):
                    w1h_s = wpool.tile([P, cfg.DC, P], F32R, name="w1h_s")
                    w1g_s = wpool.tile([P, cfg.DC, P], F32R, name="w1g_s")
                    nc.sync.dma_start(
                        out=w1h_s,
                        in_=w1hT[:, mt * P:(mt + 1) * P]
                        .rearrange("(c p) m -> p c m", p=P))
                    nc.sync.dma_start(
                        out=w1g_s,
                        in_=w1gT[:, mt * P:(mt + 1) * P]
                        .rearrange("(c p) m -> p c m", p=P))
                    zh_ps = ph.tile([P, QW], F32, name="zh_ps")
                    zg_ps = pg.tile([P, QW], F32, name="zg_ps")
                    for d in range(cfg.DC):
                        first, last = d == 0, d == cfg.DC - 1
                        nc.tensor.matmul(zh_ps, w1h_s[:, d, :], y2q[d],
                                         start=first, stop=last)
                        nc.tensor.matmul(zg_ps, w1g_s[:, d, :], y2q[d],
                                         start=first, stop=last)
                    gel = spool.tile([P, QW], F32, name="gel")
                    nc.scalar.activation(out=gel, in_=zh_ps,
                                         func=ACTF.Gelu_apprx_tanh,
                                         bias=b1h_t[:, mt:mt + 1],
                                         scale=1.0)
                    nc.vector.scalar_tensor_tensor(
                        out=uT[mt], in0=zg_ps,
                        scalar=b1g_t[:, mt:mt + 1], in1=gel,
                        op0=ALU.add, op1=ALU.mult)
                # w2: partial2 rows for this quarter (w2 streamed per dt)
                for dt in range(cfg.D // 512):
                    w2blk = wpool.tile([P, cfg.HLT, 512], F32R, name="w2blk")
                    nc.sync.dma_start(
                        out=w2blk,
                        in_=w2T[:, dt * 512:(dt + 1) * 512]
                        .rearrange("(u p) n -> p u n", p=P))
                    for ss in range(QW // P):
                        p2_ps = p2.tile([P, 512], F32, name="p2_ps")
                        for u in range(cfg.HLT):
                            nc.tensor.matmul(
                                p2_ps, uT[u][:, ss * P:(ss + 1) * P],
                                w2blk[:, u, :],
                                start=(u == 0), stop=(u == cfg.HLT - 1))
                        p2_sb = spool.tile([P, 512], F32, name="p2_sb")
                        nc.vector.tensor_copy(out=p2_sb, in_=p2_ps)
                        nc.sync.dma_start(
                            out=part_2[qi][ss * P:(ss + 1) * P,
                                           dt * 512:(dt + 1) * 512],
                            in_=p2_sb)
                nc.gpsimd.collective_compute(
                    "ReduceScatter", ALU.add, replica_groups=rg,
                    ins=[part_2[qi][:]], outs=[rs2[qi][:]])
                if qi >= 1:
                    final_chunk(qi - 1, fpool)
            final_chunk(NQ - 1, fpool)

        for pool in (x2res, consts, dram):
            pool.release()

    nc.compile()
    return nc


def _get_built(cfg: Cfg):
    if cfg not in _BUILT:
        _BUILT[cfg] = _build(cfg)
    return _BUILT[cfg]


def _row_index(cfg: Cfg, r: int) -> np.ndarray:
    """Global row indices owned by core r, in local storage order."""
    idx = []
    for c in range(cfg.ST):
        base = c * 512 + r * cfg.RW
        idx.extend(range(base, base + cfg.RW))
    return np.array(idx)


def make_in_maps(cfg: Cfg, inputs: dict) -> list:
    """Host-side sharding: full inputs -> per-core input maps."""
    f32 = np.float32
    x = np.asarray(inputs["x"], f32)
    wq = np.asarray(inputs["wq"], f32)
    wk = np.asarray(inputs["wk"], f32)
    wv = np.asarray(inputs["wv"], f32)
    wo = np.asarray(inputs["wo"], f32)
    w1 = np.asarray(inputs["w1"], f32)
    b1 = np.asarray(inputs["b1"], f32)
    w2 = np.asarray(inputs["w2"], f32)
    b2 = np.asarray(inputs["b2"], f32)
    n1w = np.asarray(inputs["n1_w"], f32)
    n1b = np.asarray(inputs["n1_b"], f32)
    n2w = np.asarray(inputs["n2_w"], f32)
    n2b = np.asarray(inputs["n2_b"], f32)

    c = np.ascontiguousarray
    maps = []
    for r in range(cfg.NC):
        ml = slice(r * cfg.ML, (r + 1) * cfg.ML)
        hl = slice(r * cfg.HL, (r + 1) * cfg.HL)
        hlg = slice(cfg.HID + r * cfg.HL, cfg.HID + (r + 1) * cfg.HL)
        maps.append({
            "x_loc": c(x[_row_index(cfg, r)]),
            "wqT": c(wq[ml].T),
            "wkT": c(wk[ml].T),
            "wvT": c(wv[ml].T),
            "woT": c(wo[:, ml].T),
            "w1hT": c(w1[hl].T),
            "w1gT": c(w1[hlg].T),
            "w2T": c(w2[:, hl].T),
            "b1h": c(b1[hl]),
            "b1g": c(b1[hlg]),
            "b2": b2,
            "n1w": n1w, "n1b": n1b, "n2w": n2w, "n2b": n2b,
        })
    return maps


def run(cfg: Cfg, inputs: dict, **kw):
    from concourse.bass_utils import run_bass_kernel_spmd
    nc = _get_built(cfg)
    in_maps = make_in_maps(cfg, inputs)
    res = run_bass_kernel_spmd(nc, in_maps, core_ids=list(range(cfg.NC)), **kw)
    out = np.empty((cfg.S, cfg.D), np.float32)
    for r in range(cfg.NC):
        out[_row_index(cfg, r)] = res.results[r]["out_loc"]
    return out, res


def kernel(**inputs) -> np.ndarray:
    out, _ = run(FULL, inputs)
    return out



# revision 21
# speedup vs baseline: 1.4524x; 1.4524x over previous
"""Trainium2 Bass kernel for a dense transformer block (pre-norm attention +
GeGLU MLP), tensor-parallel across 8 NeuronCores.

v3 design: all matmul operands, staged activations and collectives in
bfloat16 (tolerance is 2e-2; bf16 lands ~1e-3). Collectives move ROW-major
activations (4KB contiguous lines) in 128-row-per-core pairs - 9 collective
ops total (1 barrier + 2 AG1 + 2 RS1 + 2 AG2 + 2 RS2). This matters
because each ReduceScatter has a ~15us fixed cost on top of bytes, and the
NeuronCore is util-throttled to ~50% while any collective is on the wire.
Each core re-creates feature-major tiles from the gathered rows with ONE
hardware DMA-transpose per 512-row slab (bf16 XBAR path, 14ns/16x128-tile,
3D output [128, 16, 512] with d-major feature blocks) - no PE transposes,
no tiny strided DMA packets, no per-tile DMA dispatch overhead (~1us each).
RMSNorm affine params are folded into adjacent weights host-side
(w *= n1w, bias = w@n1b), so on-chip norm is a pure x*rsqrt(mean(x^2)+eps);
b2 is added host-side. MLP weights load once (the baseline re-loaded w1/w2
every quarter: ~96MB of DMA).

Row indexing: core r owns global rows {c*512 + r*64 + i}, stored in c-major
order. Pair t of a core = its local rows [t*128, (t+1)*128). AllGather of a
pair produces the 1024 rows of global slabs {2t, 2t+1} in rank-major
"position" order; every later stage (attention rows, wo partials,
ReduceScatter chunks, MLP rows, residuals, output) uses the same position
order, so all mappings are identity and reductions land back on the
owning core's contiguous local rows. Attention is order-invariant (full
mask, softmax over all keys).
"""

import sys

for _p in ("/opt/trn_rl_repo",):
    if _p not in sys.path:
        sys.path.insert(0, _p)

import math
from dataclasses import dataclass

import numpy as np


@dataclass(frozen=True)
class Cfg:
    S: int = 2048       # sequence length
    D: int = 2048       # model dim
    H: int = 16         # heads (total)
    DH: int = 128       # head dim (must be 128)
    HID: int = 8192     # GeGLU hidden (total)
    NC: int = 8         # cores
    EPS: float = 1e-5

    @property
    def P(self):
        return 128

    @property
    def SL(self):   # rows per core
        return self.S // self.NC

    @property
    def DC(self):   # d chunks of 128
        return self.D // self.P

    @property
    def MH(self):   # heads per core
        return self.H // self.NC

    @property
    def ML(self):   # local qkv features
        return self.MH * self.DH

    @property
    def HL(self):   # local hidden
        return self.HID // self.NC

    @property
    def HLT(self):  # local hidden tiles of 128
        return self.HL // self.P

    @property
    def NP(self):   # 128-row pairs per core
        return self.SL // self.P

    @property
    def SV(self):   # 512-position slabs
        return self.S // 512

    @property
    def RW(self):   # rows per (chunk, rank) in the c-major layout
        return 512 // self.NC

    @property
    def DQ(self):   # 512-wide d chunks
        return self.D // 512


FULL = Cfg()

_BUILT = {}


def _build(cfg: Cfg):
    """Build + compile the SPMD program."""
    import concourse.tile as tile
    from concourse import bacc, mybir

    P = cfg.P
    F32 = mybir.dt.float32
    F32R = mybir.dt.float32r
    BF16 = mybir.dt.bfloat16
    assert cfg.DH == P and cfg.ML == 256 and cfg.S % 1024 == 0

    nc = bacc.Bacc("TRN2", target_bir_lowering=False, debug=False,
                   num_devices=cfg.NC)

    def din(name, shape, dt=F32):
        return nc.dram_tensor(name, list(shape), dt, kind="ExternalInput").ap()

    x_loc = din("x_loc", [cfg.SL, cfg.D])
    wqT = din("wqT", [cfg.D, cfg.ML], BF16)
    wkT = din("wkT", [cfg.D, cfg.ML], BF16)
    wvT = din("wvT", [cfg.D, cfg.ML], BF16)
    woT = din("woT", [cfg.ML, cfg.D], BF16)
    w1hT = din("w1hT", [cfg.D, cfg.HL], BF16)
    w1gT = din("w1gT", [cfg.D, cfg.HL], BF16)
    w2T = din("w2T", [cfg.HL, cfg.D], BF16)
    bq = din("bq", [cfg.ML])
    bk = din("bk", [cfg.ML])
    bv = din("bv", [cfg.ML])
    b1h = din("b1h", [cfg.HL])
    b1g = din("b1g", [cfg.HL])

    out_loc = nc.dram_tensor("out_loc", [cfg.SL, cfg.D], F32,
                             kind="ExternalOutput").ap()

    rg = [list(range(cfg.NC))]
    AX = mybir.AxisListType.X
    ALU = mybir.AluOpType
    ACTF = mybir.ActivationFunctionType
    inv_sqrt_dh = 1.0 / math.sqrt(cfg.DH)

    with tile.TileContext(nc) as tc:
        # ---- internal DRAM (all pair-granular) ----
        dram = tc.alloc_tile_pool(name="dram", bufs=1, space="DRAM")
        y1r_loc = [dram.tile([P, cfg.D], BF16, name=f"y1r_loc{t}")
                   for t in range(cfg.NP)]
        y1r_ag = [dram.tile([cfg.NC, P, cfg.D], BF16,
                            name=f"y1r_ag{t}", addr_space="Shared")
                  for t in range(cfg.NP)]
        part_o = [dram.tile([cfg.NC * P, cfg.D], BF16, name=f"part_o{t}")
                  for t in range(cfg.NP)]
        rs1 = [dram.tile([P, cfg.D], BF16, name=f"rs1_{t}")
               for t in range(cfg.NP)]
        y2r_loc = [dram.tile([P, cfg.D], BF16, name=f"y2r_loc{t}")
                   for t in range(cfg.NP)]
        y2r_ag = [dram.tile([cfg.NC, P, cfg.D], BF16,
                            name=f"y2r_ag{t}", addr_space="Shared")
                  for t in range(cfg.NP)]
        part_2 = [dram.tile([cfg.NC * P, cfg.D], BF16, name=f"part_2_{t}")
                  for t in range(cfg.NP)]
        rs2 = [dram.tile([P, cfg.D], BF16, name=f"rs2_{t}")
               for t in range(cfg.NP)]

        # ---- constants / persistent small tiles ----
        consts = tc.alloc_tile_pool(name="consts", bufs=1)
        ones_col = consts.tile([P, 1], BF16, name="ones_col")
        nc.vector.memset(ones_col, 1.0)
        ones_row_f = consts.tile([1, P], F32, name="ones_row_f")
        nc.vector.memset(ones_row_f, 1.0)
        ones_row = consts.tile([1, P], F32R, name="ones_row")
        nc.vector.tensor_copy(out=ones_row, in_=ones_row_f)
        eps_t = consts.tile([P, 1], F32, name="eps_t")
        nc.vector.memset(eps_t, cfg.EPS)

        def load_pp(name, src, n):  # [n*P] dram -> [P, n] sbuf (per-partition)
            t = consts.tile([P, n], F32, name=name)
            nc.scalar.dma_start(out=t, in_=src.rearrange("(t p) -> p t", p=P))
            return t

        bq_t = load_pp("bq_t", bq, cfg.MH)
        bk_t = load_pp("bk_t", bk, cfg.MH)
        bv_t = load_pp("bv_t", bv, cfg.MH)
        b1h_t = load_pp("b1h_t", b1h, cfg.HLT)
        b1g_t = load_pp("b1g_t", b1g, cfg.HLT)

        # ---- persistent weights (all bf16) ----
        wpool = tc.alloc_tile_pool(name="weights", bufs=1)
        wq_t = wpool.tile([P, cfg.DC, cfg.ML], BF16, name="wq_t")
        wk_t = wpool.tile([P, cfg.DC, cfg.ML], BF16, name="wk_t")
        wv_t = wpool.tile([P, cfg.DC, cfg.ML], BF16, name="wv_t")
        for w_t, src in ((wq_t, wqT), (wk_t, wkT), (wv_t, wvT)):
            nc.scalar.dma_start(
                out=w_t, in_=src.rearrange("(c p) m -> p c m", p=P))
        woT_t = [wpool.tile([P, cfg.D], BF16, name=f"woT{m}")
                 for m in range(cfg.MH)]
        for m in range(cfg.MH):
            nc.scalar.dma_start(out=woT_t[m], in_=woT[m * P:(m + 1) * P, :])
        w1h_s = wpool.tile([P, cfg.DC, cfg.HL], BF16, name="w1h_s")
        w1g_s = wpool.tile([P, cfg.DC, cfg.HL], BF16, name="w1g_s")

        # residuals x2 = x + attn_out, SBUF-resident per pair
        x2res = tc.alloc_tile_pool(name="x2res", bufs=1)
        x2sb = [x2res.tile([P, cfg.D], F32, name=f"x2sb{t}")
                for t in range(cfg.NP)]

        # Transpose staging pool is shared by QKV (y1T) and MLP (y2T):
        # two [128, DC, 512] slots. All XBAR transposes are issued on the
        # scalar queue, scheduled into collective-free windows (any DMA is
        # starved while a collective is on the wire).
        tpose = tc.alloc_tile_pool(name="tpose", bufs=1)

        # persistent qkv results (released after attention)
        qkvres = tc.alloc_tile_pool(name="qkvres", bufs=1)
        qT = [qkvres.tile([P, cfg.S], BF16, name=f"qT{m}")
              for m in range(cfg.MH)]
        kT = [qkvres.tile([P, cfg.S], BF16, name=f"kT{m}")
              for m in range(cfg.MH)]
        v_sb = [qkvres.tile([P, cfg.ML], BF16, name=f"v{j}")
                for j in range(cfg.S // P)]


        def load_T(dst, ag, sub):
            """One XBAR DMA-transpose: 512 gathered rows x D feats ->
            [128, DC, 512] feature-major (d-major blocks)."""
            nc.scalar.dma_start(
                out=dst,
                in_=ag.rearrange("b r d -> (b r) d")[sub * 512:
                                                     (sub + 1) * 512, :],
                transpose=True)

        def tpose_tile(k):
            return tpose.tile([P, cfg.DC, 512], BF16, name=f"tp{k}",
                              tag=f"tp{k}")

        # ================= phase 0: norm1 + pair AG =================
        with tc.tile_pool(name="nrm1", bufs=1) as pool, \
             tc.tile_pool(name="nrm1s", bufs=2) as spool:
            y1T01 = []
            for t in range(cfg.NP):
                xt = pool.tile([P, cfg.D], F32, name="xt", tag="xt")
                nc.scalar.dma_start(out=xt,
                                    in_=x_loc[t * P:(t + 1) * P, :])
                sq = pool.tile([P, cfg.D], F32, name="sq", tag="sq")
                nc.vector.tensor_mul(sq, xt, xt)
                ssum = spool.tile([P, 1], F32, name="ssum", tag="ssum")
                nc.vector.tensor_reduce(out=ssum, in_=sq, axis=AX, op=ALU.add)
                rms = spool.tile([P, 1], F32, name="rms", tag="rms")
                nc.scalar.activation(out=rms, in_=ssum, func=ACTF.Sqrt,
                                     bias=eps_t, scale=1.0 / cfg.D)
                inv = spool.tile([P, 1], F32, name="inv", tag="inv")
                nc.vector.reciprocal(out=inv, in_=rms)
                y1r = pool.tile([P, cfg.D], BF16, name="y1r", tag="y1r")
                with nc.allow_low_precision(reason="bf16 activations"):
                    nc.vector.tensor_scalar_mul(y1r, xt, inv)
                if t == 0:
                    nc.sync.dma_start(out=y1r_loc[t], in_=y1r)
                    nc.gpsimd.collective_compute(
                        "AllGather", ALU.bypass, replica_groups=rg,
                        ins=[y1r_loc[t][:]], outs=[y1r_ag[t][:]])
                    # transpose slabs 0/1 in the window after AG1(0); the
                    # pair-1 store below is queued behind them on the same
                    # scalar queue, so AG1(1) cannot wall them off.
                    for sub in range(2):
                        tt = tpose_tile(sub)
                        load_T(tt, y1r_ag[0], sub)
                        y1T01.append(tt)
                else:
                    nc.scalar.dma_start(out=y1r_loc[t], in_=y1r)
                    nc.gpsimd.collective_compute(
                        "AllGather", ALU.bypass, replica_groups=rg,
                        ins=[y1r_loc[t][:]], outs=[y1r_ag[t][:]])

        # ================= phase 1: QKV per 512-position slab ============
        with tc.tile_pool(name="qkv_pq", bufs=1, space="PSUM") as pq, \
             tc.tile_pool(name="qkv_pk", bufs=1, space="PSUM") as pk, \
             tc.tile_pool(name="qkv_pv", bufs=1, space="PSUM") as pv:
            for sv in range(cfg.SV):
                y1T = y1T01[sv] if sv < 2 else tpose_tile(sv % 2)
                if sv >= 2:
                    load_T(y1T, y1r_ag[sv // 2], sv % 2)
                q_ps = [pq.tile([P, 512], F32, name=f"q_ps{m}")
                        for m in range(cfg.MH)]
                k_ps = [pk.tile([P, 512], F32, name=f"k_ps{m}")
                        for m in range(cfg.MH)]
                v_ps = [pv.tile([P, cfg.ML], F32, name=f"v_ps{j}")
                        for j in range(4)]
                for d in range(cfg.DC):
                    first, last = d == 0, d == cfg.DC - 1
                    for m in range(cfg.MH):
                        nc.tensor.matmul(
                            q_ps[m], wq_t[:, d, m * P:(m + 1) * P],
                            y1T[:, d, :], start=first, stop=last)
                        nc.tensor.matmul(
                            k_ps[m], wk_t[:, d, m * P:(m + 1) * P],
                            y1T[:, d, :], start=first, stop=last)
                    for ss in range(4):
                        nc.tensor.matmul(
                            v_ps[ss], y1T[:, d, ss * P:(ss + 1) * P],
                            wv_t[:, d, :], start=first, stop=last)
                sl = slice(sv * 512, (sv + 1) * 512)
                with nc.allow_low_precision(reason="bf16 activations"):
                    for m in range(cfg.MH):
                        # q/k with folded-norm bias, cast to bf16
                        nc.scalar.activation(
                            out=qT[m][:, sl], in_=q_ps[m], func=ACTF.Identity,
                            bias=bq_t[:, m:m + 1], scale=1.0)
                        nc.vector.tensor_scalar(
                            out=kT[m][:, sl], in0=k_ps[m],
                            scalar1=bk_t[:, m:m + 1], scalar2=None,
                            op0=ALU.add)
                    for ss in range(4):
                        # gpsimd can't read PSUM; split v across ACT/DVE
                        if ss < 2:
                            nc.scalar.activation(out=v_sb[sv * 4 + ss],
                                                 in_=v_ps[ss],
                                                 func=ACTF.Copy)
                        else:
                            nc.vector.tensor_copy(out=v_sb[sv * 4 + ss],
                                                  in_=v_ps[ss])

        # ======== phases 2+3: attention + wo + pair RS1/norm2/AG2 ======
        JT = cfg.S // P

        def phase3_pair(t, pool, smpool):
            r1 = pool.tile([P, cfg.D], BF16, name="r1", tag="r1")
            nc.scalar.dma_start(out=r1, in_=rs1[t][:])
            xt = pool.tile([P, cfg.D], F32, name="p3x", tag="p3x")
            nc.scalar.dma_start(out=xt, in_=x_loc[t * P:(t + 1) * P, :])
            nc.vector.tensor_add(x2sb[t], xt, r1)
            # reuse xt for x2^2 (xt is dead after the add)
            nc.gpsimd.tensor_mul(xt, x2sb[t], x2sb[t])
            ssum = smpool.tile([P, 1], F32, name="p3ss", tag="p3ss")
            nc.vector.tensor_reduce(out=ssum, in_=xt, axis=AX, op=ALU.add)
            rms = smpool.tile([P, 1], F32, name="p3rms", tag="p3rms")
            nc.scalar.activation(out=rms, in_=ssum, func=ACTF.Sqrt,
                                 bias=eps_t, scale=1.0 / cfg.D)
            inv = smpool.tile([P, 1], F32, name="p3inv", tag="p3inv")
            nc.vector.reciprocal(out=inv, in_=rms)
            y2r = pool.tile([P, cfg.D], BF16, name="y2r", tag="y2r")
            with nc.allow_low_precision(reason="bf16 activations"):
                nc.vector.tensor_scalar_mul(y2r, x2sb[t], inv)
            nc.sync.dma_start(out=y2r_loc[t], in_=y2r)
            nc.gpsimd.collective_compute(
                "AllGather", ALU.bypass, replica_groups=rg,
                ins=[y2r_loc[t][:]], outs=[y2r_ag[t][:]])

        with tc.tile_pool(name="att_ex", bufs=2) as expool, \
             tc.tile_pool(name="att_s", bufs=1) as spool, \
             tc.tile_pool(name="att_ao", bufs=1) as aopool, \
             tc.tile_pool(name="att_po", bufs=3) as popool, \
             tc.tile_pool(name="nrm2big", bufs=1) as n2pool, \
             tc.tile_pool(name="nrm2sm", bufs=2) as n2sm, \
             tc.tile_pool(name="att_pqk", bufs=2, space="PSUM") as pqk, \
             tc.tile_pool(name="att_pav", bufs=2, space="PSUM") as pav, \
             tc.tile_pool(name="att_psb", bufs=1, space="PSUM") as psb, \
             tc.tile_pool(name="att_ppo", bufs=2, space="PSUM") as ppo:
            for sv in range(cfg.SV):
                sl = slice(sv * 512, (sv + 1) * 512)
                aoT_sl = [aopool.tile([P, 512], BF16, name=f"aoT{h}",
                                      tag=f"aoT{h}") for h in range(cfg.MH)]
                for h in range(cfg.MH):
                    av_ps = pav.tile([P, 512], F32, name="av_ps")
                    sum_ps = psb.tile([1, 512], F32, name="sum_ps", tag="sum")
                    exs = []
                    for j in range(JT):
                        qk_ps = pqk.tile([P, 512], F32, name="qk_ps")
                        nc.tensor.matmul(qk_ps, kT[h][:, j * P:(j + 1) * P],
                                         qT[h][:, sl], start=True, stop=True)
                        if exs:  # software pipeline: PE j, ACT exps j-1
                            ex_p = exs.pop()
                            nc.tensor.matmul(sum_ps, ones_col, ex_p,
                                             start=(j == 1), stop=False)
                            nc.tensor.matmul(
                                av_ps, v_sb[j - 1][:, h * P:(h + 1) * P],
                                ex_p, start=(j == 1), stop=False)
                        ex = expool.tile([P, 512], BF16, name="ex")
                        with nc.allow_low_precision(reason="bf16 softmax"):
                            nc.scalar.activation(out=ex, in_=qk_ps,
                                                 func=ACTF.Exp,
                                                 scale=inv_sqrt_dh)
                        exs.append(ex)
                    ex_p = exs.pop()
                    nc.tensor.matmul(sum_ps, ones_col, ex_p,
                                     start=False, stop=True)
                    nc.tensor.matmul(av_ps,
                                     v_sb[JT - 1][:, h * P:(h + 1) * P],
                                     ex_p, start=False, stop=True)
                    rec = spool.tile([1, 512], F32R, name="rec")
                    with nc.allow_low_precision(
                            reason="softmax denom reciprocal in f32r"):
                        nc.vector.reciprocal(out=rec, in_=sum_ps)
                    bc_ps = psb.tile([P, 512], F32, name="bc_ps", tag="bc")
                    nc.tensor.matmul(bc_ps, ones_row, rec,
                                     start=True, stop=True)
                    bc_sb = spool.tile([P, 512], F32, name="bc_sb")
                    nc.scalar.activation(out=bc_sb, in_=bc_ps, func=ACTF.Copy)
                    nc.vector.tensor_mul(bc_sb, av_ps, bc_sb)
                    with nc.allow_low_precision(reason="bf16 activations"):
                        nc.gpsimd.tensor_scalar(
                            out=aoT_sl[h], in0=bc_sb,
                            scalar1=bv_t[:, h:h + 1], scalar2=None,
                            op0=ALU.add)
                # wo for this slab -> positions of pair sv//2
                pbase = (sv % 2) * 512
                for ss in range(4):
                    po_sb = popool.tile([P, cfg.D], BF16, name="po_sb",
                                        tag="po_sb")
                    for dtq in range(cfg.DQ):
                        po_ps = ppo.tile([P, 512], F32, name="po_ps")
                        for m in range(cfg.MH):
                            nc.tensor.matmul(
                                po_ps, aoT_sl[m][:, ss * P:(ss + 1) * P],
                                woT_t[m][:, dtq * 512:(dtq + 1) * 512],
                                start=(m == 0), stop=(m == cfg.MH - 1))
                        with nc.allow_low_precision(reason="bf16 partials"):
                            nc.vector.tensor_copy(
                                out=po_sb[:, dtq * 512:(dtq + 1) * 512],
                                in_=po_ps)
                    nc.sync.dma_start(
                        out=part_o[sv // 2][pbase + ss * P:
                                            pbase + (ss + 1) * P, :],
                        in_=po_sb)
                if sv % 2 == 1:
                    nc.gpsimd.collective_compute(
                        "ReduceScatter", ALU.add, replica_groups=rg,
                        ins=[part_o[sv // 2][:]], outs=[rs1[sv // 2][:]])
                if sv == 0:
                    # w1 weights are first needed in phase 4; issue their
                    # DMA now so it overlaps the attention phase.
                    nc.scalar.dma_start(
                        out=w1h_s,
                        in_=w1hT.rearrange("(c p) m -> p c m", p=P))
                    nc.scalar.dma_start(
                        out=w1g_s,
                        in_=w1gT.rearrange("(c p) m -> p c m", p=P))
                if sv >= 2 and sv % 2 == 0:
                    phase3_pair(sv // 2 - 1, n2pool, n2sm)
                    if sv == 2:
                        # prefetch MLP half-0 transposes into the
                        # collective-free window after AG2(0)
                        y2T0 = [tpose_tile(sub) for sub in range(2)]
                        for sub in range(2):
                            load_T(y2T0[sub], y2r_ag[0], sub)
            phase3_pair(cfg.NP - 1, n2pool, n2sm)
            if cfg.NP == 1:  # mini: pair 0 is the last pair
                y2T0 = [tpose_tile(sub) for sub in range(2)]
                for sub in range(2):
                    load_T(y2T0[sub], y2r_ag[0], sub)
        qkvres.release()

        # ============ phase 4: MLP per 1024-row half + RS2 + final ========
        def final_pair(t, pool):
            r2 = pool.tile([P, cfg.D], BF16, name="r2", tag="r2")
            nc.scalar.dma_start(out=r2, in_=rs2[t][:])
            o_t = pool.tile([P, cfg.D], F32, name="o_t", tag="o_t")
            nc.vector.tensor_add(o_t, x2sb[t], r2)
            nc.sync.dma_start(out=out_loc[t * P:(t + 1) * P, :], in_=o_t)

        with tc.tile_pool(name="mlp_u", bufs=1) as upool, \
             tc.tile_pool(name="mlp_w2", bufs=2) as w2pool, \
             tc.tile_pool(name="mlp_gel", bufs=1) as gpool, \
             tc.tile_pool(name="mlp_p2sb", bufs=1) as p2sbp, \
             tc.tile_pool(name="fin", bufs=1) as fpool, \
             tc.tile_pool(name="mlp_ph", bufs=2, space="PSUM") as ph, \
             tc.tile_pool(name="mlp_pg", bufs=2, space="PSUM") as pg, \
             tc.tile_pool(name="mlp_p2", bufs=3, space="PSUM") as p2:
            y2T_next = y2T0
            for ht in range(cfg.NP):
                y2T = y2T_next
                uT = [upool.tile([P, 512], BF16, name=f"uT{i}", tag=f"uT{i}")
                      for i in range(2 * cfg.HLT)]
                for sub in range(2):
                    for mt in range(cfg.HLT):
                        zh_ps = ph.tile([P, 512], F32, name="zh_ps")
                        zg_ps = pg.tile([P, 512], F32, name="zg_ps")
                        for d in range(cfg.DC):
                            first, last = d == 0, d == cfg.DC - 1
                            nc.tensor.matmul(
                                zh_ps, w1h_s[:, d, mt * P:(mt + 1) * P],
                                y2T[sub][:, d, :], start=first, stop=last)
                            nc.tensor.matmul(
                                zg_ps, w1g_s[:, d, mt * P:(mt + 1) * P],
                                y2T[sub][:, d, :], start=first, stop=last)
                        gel = gpool.tile([P, 512], F32, name="gel", tag="gel")
                        nc.scalar.activation(out=gel, in_=zh_ps,
                                             func=ACTF.Gelu_apprx_tanh,
                                             bias=b1h_t[:, mt:mt + 1],
                                             scale=1.0)
                        with nc.allow_low_precision(reason="bf16 acts"):
                            nc.vector.scalar_tensor_tensor(
                                out=uT[sub * cfg.HLT + mt], in0=zg_ps,
                                scalar=b1g_t[:, mt:mt + 1], in1=gel,
                                op0=ALU.add, op1=ALU.mult)
                if ht + 1 < cfg.NP:
                    # prefetch next half's transposes now: the w2 section
                    # below keeps the PE busy while they run, and no
                    # collective is on the wire yet (RS2(ht) waits on the
                    # part_2 stores)
                    y2T_next = [tpose_tile(sub) for sub in range(2)]
                    for sub in range(2):
                        load_T(y2T_next[sub], y2r_ag[ht + 1], sub)
                # w2: partial rows for this half; one [128, 8, 512] staging
                # tile per dtq -> single batched DMA into part_2's column
                # block (row ss*128+p, col dtq*512+n)
                NSS = cfg.NC * P // 128  # 128-row blocks per half
                for dtq in range(cfg.DQ):
                    w2blk = w2pool.tile([P, cfg.HLT, 512], BF16,
                                        name="w2blk", tag="w2blk")
                    nc.scalar.dma_start(
                        out=w2blk,
                        in_=w2T[:, dtq * 512:(dtq + 1) * 512]
                        .rearrange("(u p) n -> p u n", p=P))
                    p2_sb = p2sbp.tile([P, NSS, 512], BF16, name="p2_sb",
                                       tag="p2_sb")
                    for ss in range(NSS):
                        sub, ssl = ss // 4, ss % 4
                        p2_ps = p2.tile([P, 512], F32, name="p2_ps")
                        for u in range(cfg.HLT):
                            nc.tensor.matmul(
                                p2_ps,
                                uT[sub * cfg.HLT + u][:, ssl * P:
                                                      (ssl + 1) * P],
                                w2blk[:, u, :],
                                start=(u == 0), stop=(u == cfg.HLT - 1))
                        with nc.allow_low_precision(reason="bf16 partials"):
                            nc.vector.tensor_copy(out=p2_sb[:, ss, :],
                                                  in_=p2_ps)
                    nc.sync.dma_start(
                        out=part_2[ht][:, dtq * 512:(dtq + 1) * 512]
                        .rearrange("(s p) n -> p s n", p=P),
                        in_=p2_sb)
                nc.gpsimd.collective_compute(
                    "ReduceScatter", ALU.add, replica_groups=rg,
                    ins=[part_2[ht][:]], outs=[rs2[ht][:]])
                if ht >= 1:
                    final_pair(ht - 1, fpool)
            final_pair(cfg.NP - 1, fpool)

        for pool in (tpose, x2res, wpool, consts, dram):
            pool.release()

    nc.compile()
    return nc


def _get_built(cfg: Cfg):
    if cfg not in _BUILT:
        _BUILT[cfg] = _build(cfg)
    return _BUILT[cfg]


def _row_index(cfg: Cfg, r: int) -> np.ndarray:
    """Global row indices owned by core r, in local storage order."""
    idx = []
    for c in range(cfg.S // 512):
        base = c * 512 + r * cfg.RW
        idx.extend(range(base, base + cfg.RW))
    return np.array(idx)


def make_in_maps(cfg: Cfg, inputs: dict) -> list:
    """Host-side sharding: full inputs -> per-core input maps.

    RMSNorm affine params are folded into the adjacent projection
    weights: y = (x*inv)*nw + nb, so q = (x*inv) @ (nw*wq)^T + wq@nb.
    """
    import ml_dtypes
    f32 = np.float32
    bf16 = ml_dtypes.bfloat16
    x = np.asarray(inputs["x"], f32)
    wq = np.asarray(inputs["wq"], f32)
    wk = np.asarray(inputs["wk"], f32)
    wv = np.asarray(inputs["wv"], f32)
    wo = np.asarray(inputs["wo"], f32)
    w1 = np.asarray(inputs["w1"], f32)
    b1 = np.asarray(inputs["b1"], f32)
    w2 = np.asarray(inputs["w2"], f32)
    n1w = np.asarray(inputs["n1_w"], f32)
    n1b = np.asarray(inputs["n1_b"], f32)
    n2w = np.asarray(inputs["n2_w"], f32)
    n2b = np.asarray(inputs["n2_b"], f32)

    c = np.ascontiguousarray
    maps = []
    for r in range(cfg.NC):
        ml = slice(r * cfg.ML, (r + 1) * cfg.ML)
        hl = slice(r * cfg.HL, (r + 1) * cfg.HL)
        hlg = slice(cfg.HID + r * cfg.HL, cfg.HID + (r + 1) * cfg.HL)
        wq_s, wk_s, wv_s = wq[ml], wk[ml], wv[ml]
        w1h_sh, w1g_sh = w1[hl], w1[hlg]
        maps.append({
            "x_loc": c(x[_row_index(cfg, r)]),
            "wqT": c((wq_s * n1w[None, :]).T.astype(bf16)),
            "wkT": c((wk_s * n1w[None, :]).T.astype(bf16)),
            "wvT": c((wv_s * n1w[None, :]).T.astype(bf16)),
            "woT": c(wo[:, ml].T.astype(bf16)),
            "w1hT": c((w1h_sh * n2w[None, :]).T.astype(bf16)),
            "w1gT": c((w1g_sh * n2w[None, :]).T.astype(bf16)),
            "w2T": c(w2[:, hl].T.astype(bf16)),
            "bq": c(wq_s @ n1b),
            "bk": c(wk_s @ n1b),
            "bv": c(wv_s @ n1b),
            "b1h": c(b1[hl] + w1h_sh @ n2b),
            "b1g": c(b1[hlg] + w1g_sh @ n2b),
        })
    return maps


def run(cfg: Cfg, inputs: dict, **kw):
    from concourse.bass_utils import run_bass_kernel_spmd
    nc = _get_built(cfg)
    in_maps = make_in_maps(cfg, inputs)
    res = run_bass_kernel_spmd(nc, in_maps, core_ids=list(range(cfg.NC)), **kw)
    b2 = np.asarray(inputs["b2"], np.float32)
    out = np.empty((cfg.S, cfg.D), np.float32)
    for r in range(cfg.NC):
        out[_row_index(cfg, r)] = res.results[r]["out_loc"]
    out += b2[None, :]
    return out, res


def kernel(**inputs) -> np.ndarray:
    out, _ = run(FULL, inputs)
    return out
